# revision 7
# baseline (speedup 1.0000x reference)
"""Multi-query sparse attention (causal + rel-pos-bias + XL memory) on 8 TRN2 cores.

Sharding: queries are sharded across cores. Core c handles query blocks
A=[64c, 64c+64) and B=[64(15-c), 64(15-c)+64) for both batch elements --
the A/B pairing balances causal work. K/V (single shared head) are computed
from each core's own rows and AllGathered (bf16). rel_pos_bias is sharded by
query rows, host-transposed to [keys, head, query] layout with the causal
mask baked in as -3e38 (exp -> 0), so the device computes sim directly in
[keys, head*query] layout and attn@v consumes it without any transpose.
Softmax uses exp(sim)*exp(bias) (no max subtraction -- values are small) and
a ones-column in v to get the normalizer from the same matmul. All large
matmuls run in bf16 (fp32 matmul is half-rate LOW_HIGH on trn2); PSUM
accumulation stays fp32. Heads are processed in pairs (even/odd stacked on
partitions) so q/out projections contract over K=128.

Head storage order within each group of 8: [evens, odds] so pair members
land at free offsets t and 4+t (host permutes the bias head dim to match).
"""

import numpy as np

B, N, DIM = 2, 1024, 1024
H, DH = 16, 64
MEM = 512
J = N + MEM  # 1536
NC = 8
SCALE = DH ** -0.5
NEG = -3.0e38

_CACHE = {}

# head-slot permutation: slot i holds head HEAD_PERM[i]
HEAD_PERM = []
for g in range(2):
    HEAD_PERM += [8 * g + 2 * t for t in range(4)] + [8 * g + 2 * t + 1 for t in range(4)]


def _patch_tile_drain():
    """This walrus build only allows one sync-wait per CTRL instruction; the
    stock TileContext final drain carries several. Split them into
    single-wait nops."""
    from concourse import tile
    from concourse.vector_clock import ScopedClock, VectorClock

    if getattr(tile.TileContext, "_drain_patched", False):
        return

    def _drain_and_barrier(self, tick_clock, wait_clock):
        g = tick_clock.global_clock
        n = len(g)
        for p in range(n):
            if g[p] > 0:
                partial = VectorClock([g[i] if i == p else 0 for i in range(n)])
                nop_inst = self.nc.sync.nop()
                wait_clock.add_sem_waits(nop_inst.ins, ScopedClock({None: partial}))
        self.nc.sync.drain()
        self.nc.all_engine_barrier()
        assert self.sems is not None
        popped = self.nc._tile_sem_poison_stack.pop()
        assert popped is self._sem_poison
        self.nc.clear_and_free_semaphores(list(self.sems.allocated().values()))
        self.nc.all_engine_barrier()

    tile.TileContext._drain_and_barrier = _drain_and_barrier
    tile.TileContext._drain_patched = True


def _split_multiwait(nc, mybir):
    """Walrus here allows only one sync-wait per instruction: hoist extra
    waits onto same-engine nops placed immediately before."""
    k = 0
    for bb in nc.main_func.blocks:
        newl = []
        changed = False
        for inst in bb.instructions:
            si = inst.sync_info
            if si is not None and si.on_wait and len(si.on_wait) > 1:
                waits = list(si.on_wait)
                for w in waits[:-1]:
                    nop = mybir.InstNoOp(name=f"wsplit-{k}", ins=[], outs=[])
                    k += 1
                    nop.engine = inst.engine
                    nop.sync_info = mybir.SyncInfo(on_wait=[w], on_update=[])
                    newl.append(nop)
                si.on_wait = [waits[-1]]
                changed = True
            newl.append(inst)
        if changed:
            bb.instructions[:] = newl
            assert len(bb.instructions) == len(newl), "bb.instructions not mutable"


def _build():
    import concourse.bass as bass
    from concourse import mybir, tile

    _patch_tile_drain()
    f32 = mybir.dt.float32
    bf16 = mybir.dt.bfloat16
    AO = mybir.AluOpType
    AF = mybir.ActivationFunctionType

    nc = bass.Bass(target_bir_lowering=False)

    # ---- per-core dram parameters ----
    x_own = nc.declare_dram_parameter("x_own", [4 * 64, DIM], f32, isOutput=False)
    biasT_a = nc.declare_dram_parameter("biasT_a", [N, 2, 8, 64], f32, isOutput=False)
    biasT_b = nc.declare_dram_parameter("biasT_b", [J, 2, 8, 64], f32, isOutput=False)
    xl_k = nc.declare_dram_parameter("xl_k", [B, MEM, DH], f32, isOutput=False)
    xl_v = nc.declare_dram_parameter("xl_v", [B, MEM, DH], f32, isOutput=False)
    wq_d = nc.declare_dram_parameter("Wq", [DIM, DIM], f32, isOutput=False)
    wkv_d = nc.declare_dram_parameter("Wkv", [DIM, 2 * DH], f32, isOutput=False)
    wout_d = nc.declare_dram_parameter("Wout", [DIM, DIM], f32, isOutput=False)
    ident_d = nc.declare_dram_parameter("ident", [128, 128], f32, isOutput=False)
    identb_d = nc.declare_dram_parameter("identb", [128, 128], bf16, isOutput=False)
    ones_d = nc.declare_dram_parameter("ones", [128, 64], bf16, isOutput=False)
    bout_d = nc.declare_dram_parameter("bout_bc", [64, DIM], f32, isOutput=False)

    out_part = nc.declare_dram_parameter("out_part", [B, 2, 64, DIM], f32, isOutput=True)
    nxl_part = nc.declare_dram_parameter("nxl_part", [B, 64, 2, DH], f32, isOutput=True)

    NKT = (8, 12)  # key 128-chunks for query block A (keys<1024), B (keys<1536)

    with tile.TileContext(nc, num_cores=NC) as tc:
        import contextlib

        ctx = contextlib.ExitStack()
        with ctx:
            per = ctx.enter_context(tc.tile_pool(name="persist", bufs=1))
            dram = ctx.enter_context(tc.tile_pool(name="dram", bufs=1, space="DRAM"))

            # ---------- persistent SBUF ----------
            ident = per.tile([128, 128], f32, tag="ident")
            nc.sync.dma_start(out=ident[:], in_=ident_d[:])
            identb = per.tile([128, 128], bf16, tag="identb")
            nc.sync.dma_start(out=identb[:], in_=identb_d[:])
            ones_sb = per.tile([128, 64], bf16, tag="ones")
            nc.sync.dma_start(out=ones_sb[:], in_=ones_d[:])
            bout_sb = per.tile([64, DIM], f32, tag="bout")
            nc.sync.dma_start(out=bout_sb[:], in_=bout_d[:])
            wout_bf = [per.tile([128, DIM], bf16, name=f"woutb{j}", tag=f"woutb{j}") for j in range(8)]
            qT = per.tile([64, H, 256], bf16, name="qT", tag="qT")
            kT = [per.tile([64, J], bf16, name=f"kT{b}", tag=f"kT{b}") for b in range(B)]
            v_aug = [
                [per.tile([128, DH + 1], bf16, name=f"vaug{b}_{j}", tag=f"vaug{b}_{j}") for j in range(12)]
                for b in range(B)
            ]
            outT2 = [per.tile([128, 8, 64], bf16, name=f"outT2{b}", tag=f"outT2{b}") for b in range(B)]

            kv_bounce = dram.tile([128, 256], bf16, tag="kv_bounce")
            gathered = dram.tile([NC * 128, 256], bf16, tag="gathered", addr_space="Shared")

            # ---------- phase 1: x -> xT, projections ----------
            with tc.tile_pool(name="proj", bufs=1) as proj, tc.tile_pool(
                name="ppsum", bufs=2, space="PSUM"
            ) as ppsum:
                # Wout load f32 then cast to bf16 pair tiles [128, 1024]
                for j in range(8):
                    t = proj.tile([128, DIM], f32, tag="wof")
                    nc.sync.dma_start(out=t[:], in_=wout_d[128 * j : 128 * j + 128, :])
                    nc.vector.tensor_copy(out=wout_bf[j][:], in_=t[:])
                wq_bf = []
                for kt in range(8):
                    t = proj.tile([128, DIM], f32, tag=f"wqf{kt}")
                    nc.sync.dma_start(out=t[:], in_=wq_d[128 * kt : 128 * kt + 128, :])
                    tb = proj.tile([128, DIM], bf16, tag=f"wqb{kt}")
                    # fold the attention scale into Wq during the cast
                    nc.vector.tensor_scalar_mul(tb[:], t[:], SCALE)
                    wq_bf.append(tb)
                wkv_sb = []
                for kt in range(8):
                    t = proj.tile([128, 2 * DH], f32, tag=f"wkv{kt}")
                    nc.sync.dma_start(out=t[:], in_=wkv_d[128 * kt : 128 * kt + 128, :])
                    wkv_sb.append(t)
                x_sb = []
                for rt in range(2):
                    t = proj.tile([128, DIM], f32, tag=f"x{rt}")
                    nc.sync.dma_start(out=t[:], in_=x_own[128 * rt : 128 * rt + 128, :])
                    x_sb.append(t)
                xT_sb = [proj.tile([128, 256], f32, name=f"xT{kt}", tag=f"xT{kt}") for kt in range(8)]
                xT_bf = [proj.tile([128, 256], bf16, name=f"xTb{kt}", tag=f"xTb{kt}") for kt in range(8)]
                for rt in range(2):
                    for kt in range(8):
                        pt = ppsum.tile([128, 128], f32, tag="tp")
                        nc.tensor.transpose(
                            pt[:], x_sb[rt][:, 128 * kt : 128 * kt + 128], ident[:]
                        )
                        nc.vector.tensor_copy(
                            out=xT_sb[kt][:, 128 * rt : 128 * rt + 128], in_=pt[:]
                        )
                        nc.vector.tensor_copy(
                            out=xT_bf[kt][:, 128 * rt : 128 * rt + 128], in_=pt[:]
                        )

                # kv projection fp32 (feeds new_xl exactly); bf16 copy for gather
                pkv = ppsum.tile([128, 256], f32, tag="pkv")
                for kt in range(8):
                    nc.tensor.matmul(
                        pkv[:], wkv_sb[kt][:], xT_sb[kt][:], start=(kt == 0), stop=(kt == 7)
                    )
                kv_sb = proj.tile([128, 256], f32, tag="kv")
                nc.vector.tensor_copy(out=kv_sb[:], in_=pkv[:])
                kv_bf = proj.tile([128, 256], bf16, tag="kvb")
                nc.vector.tensor_copy(out=kv_bf[:], in_=pkv[:])
                nc.sync.dma_start(out=kv_bounce[:], in_=kv_bf[:])

                # new_xl output: own B-block k,v (cols 64+128b) transposed to rows
                for b in range(B):
                    pnx = ppsum.tile([64, 2, 64], f32, tag="pnx")
                    nc.tensor.transpose(
                        pnx[:], kv_sb[:, 64 + 128 * b : 128 + 128 * b], ident[:]
                    )
                    nx_sb = proj.tile([64, 2, 64], f32, tag="nx")
                    nc.vector.tensor_copy(out=nx_sb[:], in_=pnx[:])
                    nc.sync.dma_start(out=nxl_part[b], in_=nx_sb[:])

                # q projection in head pairs (K=128), scale pre-folded into Wq.
                # pq partitions: [0:64]=head 2hp dh, [64:128]=head 2hp+1 dh
                for hp in range(8):
                    pq = ppsum.tile([128, 256], f32, tag="pq")
                    for kt in range(8):
                        nc.tensor.matmul(
                            pq[:],
                            wq_bf[kt][:, 128 * hp : 128 * hp + 128],
                            xT_bf[kt][:],
                            start=(kt == 0),
                            stop=(kt == 7),
                        )
                    g, t = hp // 4, hp % 4
                    nc.vector.tensor_copy(out=qT[:, 8 * g + t, :], in_=pq[0:64])
                    nc.vector.tensor_copy(out=qT[:, 8 * g + 4 + t, :], in_=pq[64:128])

            # ---------- phase 2: AllGather k/v (bf16) ----------
            nc.gpsimd.collective_compute(
                "AllGather",
                mybir.AluOpType.bypass,
                replica_groups=[list(range(NC))],
                ins=[kv_bounce[:]],
                outs=[gathered[:]],
            )

            # ---------- phase 3: assemble kT and v_aug per batch ----------
            with tc.tile_pool(name="asm", bufs=3) as asm, tc.tile_pool(
                name="apsum", bufs=2, space="PSUM"
            ) as apsum:
                for b in range(B):
                    # xl part of kT: transpose f32 chunks, cast on copy
                    for j in range(4):
                        t = asm.tile([128, DH], f32, tag="xlk")
                        nc.sync.dma_start(out=t[:], in_=xl_k[b, 128 * j : 128 * j + 128, :])
                        pt = apsum.tile([64, 128], f32, tag="pxlk")
                        nc.tensor.transpose(pt[:], t[:], ident[:])
                        nc.vector.tensor_copy(
                            out=kT[b][:, 128 * j : 128 * j + 128], in_=pt[:]
                        )
                    # x part of kT straight from gathered (already [dh, rows], bf16)
                    for m in range(16):
                        cm = m if m < 8 else 15 - m
                        off = (0 if m < 8 else 64) + 128 * b
                        nc.sync.dma_start(
                            out=kT[b][:, 512 + 64 * m : 576 + 64 * m],
                            in_=gathered[128 * cm : 128 * cm + 64, off : off + 64],
                        )
                    # v_aug: xl chunks f32 -> cast copy
                    for j in range(4):
                        t = asm.tile([128, DH], f32, tag="xlv")
                        nc.sync.dma_start(out=t[:], in_=xl_v[b, 128 * j : 128 * j + 128, :])
                        nc.vector.tensor_copy(out=v_aug[b][j][:, 0:DH], in_=t[:])
                        nc.sync.dma_start(
                            out=v_aug[b][j][:, DH : DH + 1], in_=ones_d[:, 0:1]
                        )
                    # v_aug: x chunks via bf16 transpose of gathered vT
                    for j in range(4, 12):
                        vt = asm.tile([64, 128], bf16, tag="vt")
                        for mm in range(2):
                            m = 2 * (j - 4) + mm
                            cm = m if m < 8 else 15 - m
                            off = (0 if m < 8 else 64) + 128 * b
                            nc.sync.dma_start(
                                out=vt[:, 64 * mm : 64 * mm + 64],
                                in_=gathered[128 * cm + 64 : 128 * cm + 128, off : off + 64],
                            )
                        pv = apsum.tile([128, 64], bf16, tag="pv")
                        nc.tensor.transpose(pv[:], vt[:], identb[0:64, 0:64])
                        nc.vector.tensor_copy(out=v_aug[b][j][:, 0:DH], in_=pv[:])
                        nc.sync.dma_start(
                            out=v_aug[b][j][:, DH : DH + 1], in_=ones_d[:, 0:1]
                        )

            # ---------- phase 4: attention + output projection ----------
            biasT = (biasT_a, biasT_b)
            with tc.tile_pool(name="att", bufs=8) as att, tc.tile_pool(
                name="bias", bufs=3
            ) as biasp, tc.tile_pool(name="spsum", bufs=2, space="PSUM") as spsum, tc.tile_pool(
                name="opsum", bufs=1, space="PSUM"
            ) as opsum, tc.tile_pool(name="fpsum", bufs=1, space="PSUM") as fpsum, tc.tile_pool(
                name="small", bufs=4
            ) as small, tc.tile_pool(name="outp", bufs=4) as outp:
                for qbi in range(2):
                    nkt = NKT[qbi]
                    po = [
                        [opsum.tile([65, 8, 64], f32, name=f"po{b}_{g}", tag=f"po{b}_{g}") for g in range(2)]
                        for b in range(B)
                    ]
                    for kt in range(nkt):
                        bt = biasp.tile([128, 2, 8, 64], f32, tag="bias")
                        nc.sync.dma_start(
                            out=bt[:], in_=biasT[qbi][128 * kt : 128 * kt + 128]
                        )
                        eb = [att.tile([128, 8, 64], bf16, name=f"eb{g}", tag="ebias") for g in range(2)]
                        for g in range(2):
                            nc.scalar.activation(out=eb[g][:], in_=bt[:, g], func=AF.Exp)
                        for b in range(B):
                            qc = 64 * (2 * b + qbi)
                            for g in range(2):
                                ps = spsum.tile([128, 8, 64], f32, tag="ps")
                                nc.tensor.matmul(
                                    ps[:],
                                    kT[b][:, 128 * kt : 128 * kt + 128],
                                    qT[:, 8 * g : 8 * g + 8, qc : qc + 64],
                                    start=True,
                                    stop=True,
                                )
                                at = att.tile([128, 8, 64], bf16, tag="attn")
                                nc.scalar.activation(out=at[:], in_=ps[:], func=AF.Exp)
                                nc.vector.scalar_tensor_tensor(
                                    out=at[:], in0=at[:], scalar=1.0, in1=eb[g][:],
                                    op0=AO.mult, op1=AO.mult,
                                )
                                nc.tensor.matmul(
                                    po[b][g][:],
                                    v_aug[b][kt][:],
                                    at[:],
                                    start=(kt == 0),
                                    stop=(kt == nkt - 1),
                                )
                    # normalize: row 64 of po is Z; 1/Z = exp(-ln Z); broadcast
                    # via K=1 matmul; write bf16 head-pair-stacked outT2
                    for b in range(B):
                        for g in range(2):
                            lnz = small.tile([1, 8, 64], f32, tag="lnz")
                            nc.scalar.activation(out=lnz[:], in_=po[b][g][64:65], func=AF.Ln)
                            rz = small.tile([1, 8, 64], bf16, tag="rz")
                            nc.scalar.activation(out=rz[:], in_=lnz[:], func=AF.Exp, scale=-1.0)
                            rzp = fpsum.tile([64, 8, 64], f32, tag="pb")
                            nc.tensor.matmul(
                                rzp[:], ones_sb[0:1, :], rz[:], start=True, stop=True
                            )
                            rzb = small.tile([64, 8, 64], f32, tag="rzb")
                            nc.vector.tensor_copy(out=rzb[:], in_=rzp[:])
                            nc.vector.scalar_tensor_tensor(
                                out=outT2[b][0:64, 4 * g : 4 * g + 4, :],
                                in0=po[b][g][0:64, 0:4, :], scalar=1.0,
                                in1=rzb[0:64, 0:4, :], op0=AO.mult, op1=AO.mult,
                            )
                            nc.vector.scalar_tensor_tensor(
                                out=outT2[b][64:128, 4 * g : 4 * g + 4, :],
                                in0=po[b][g][0:64, 4:8, :], scalar=1.0,
                                in1=rzb[0:64, 4:8, :], op0=AO.mult, op1=AO.mult,
                            )
                    # final projection (K=128 head pairs) + bias
                    for b in range(B):
                        for nh in range(2):
                            pf = fpsum.tile([64, 512], f32, tag="pf")
                            for j in range(8):
                                nc.tensor.matmul(
                                    pf[:],
                                    outT2[b][:, j, :],
                                    wout_bf[j][:, 512 * nh : 512 * nh + 512],
                                    start=(j == 0),
                                    stop=(j == 7),
                                )
                            ot = outp.tile([64, 512], f32, tag="ot")
                            nc.vector.scalar_tensor_tensor(
                                out=ot[:], in0=pf[:], scalar=1.0,
                                in1=bout_sb[:, 512 * nh : 512 * nh + 512],
                                op0=AO.mult, op1=AO.add,
                            )
                            nc.sync.dma_start(
                                out=out_part[b, qbi, :, 512 * nh : 512 * nh + 512],
                                in_=ot[:],
                            )
    _split_multiwait(nc, mybir)
    return nc


def _shard(inputs):
    from concourse import mybir

    bfdt = mybir.dt.np(mybir.dt.bfloat16)
    x = np.asarray(inputs["x"], dtype=np.float32)
    xlm = np.asarray(inputs["xl_memory"], dtype=np.float32)
    bias = np.asarray(inputs["rel_pos_bias"], dtype=np.float32)
    Wq = np.ascontiguousarray(np.asarray(inputs["Wq"], dtype=np.float32))
    Wkv = np.ascontiguousarray(np.asarray(inputs["Wkv"], dtype=np.float32))
    Wout = np.ascontiguousarray(np.asarray(inputs["Wout"], dtype=np.float32))
    bout = np.asarray(inputs["bout"], dtype=np.float32)

    ident = np.eye(128, dtype=np.float32)
    identb = np.eye(128, dtype=np.float32).astype(bfdt)
    ones = np.ones((128, 64), dtype=np.float32).astype(bfdt)
    bout_bc = np.ascontiguousarray(np.broadcast_to(bout, (64, DIM)))
    xl_k = np.ascontiguousarray(xlm[:, :, 0, :])
    xl_v = np.ascontiguousarray(xlm[:, :, 1, :])

    jj = np.arange(J)[:, None]  # keys (concat space)
    rr = np.arange(64)[None, :]

    in_maps = []
    for c in range(NC):
        qsA, qsB = 64 * c, 64 * (15 - c)
        x_own = np.concatenate(
            [x[0, qsA : qsA + 64], x[0, qsB : qsB + 64],
             x[1, qsA : qsA + 64], x[1, qsB : qsB + 64]], axis=0,
        )
        bT = []
        for qs, klen in ((qsA, N), (qsB, J)):
            bb = bias[:, qs : qs + 64, :klen]  # [16, 64, klen]
            bb = np.transpose(bb, (2, 0, 1)).copy()  # [klen, 16, 64]
            bb = bb[:, HEAD_PERM, :]  # head-slot order [evens, odds] per group
            m = jj[:klen] > (qs + rr + 512)  # [klen, 64] causal+pad mask
            bb[m[:, None, :].repeat(H, axis=1)] = NEG
            bT.append(np.ascontiguousarray(bb.reshape(klen, 2, 8, 64)))
        in_maps.append(
            {
                "x_own": np.ascontiguousarray(x_own),
                "biasT_a": bT[0],
                "biasT_b": bT[1],
                "xl_k": xl_k,
                "xl_v": xl_v,
                "Wq": Wq,
                "Wkv": Wkv,
                "Wout": Wout,
                "ident": ident,
                "identb": identb,
                "ones": ones,
                "bout_bc": bout_bc,
            }
        )
    return in_maps


def _unshard(results):
    out = np.zeros((B, N, DIM), dtype=np.float32)
    new_xl = np.zeros((B, MEM, 2, DH), dtype=np.float32)
    for c in range(NC):
        qsA, qsB = 64 * c, 64 * (15 - c)
        op = results[c]["out_part"]
        out[:, qsA : qsA + 64] = op[:, 0]
        out[:, qsB : qsB + 64] = op[:, 1]
        new_xl[:, qsB - 512 : qsB - 512 + 64] = results[c]["nxl_part"]
    return out, new_xl


def kernel(**inputs):
    from concourse.bass_utils import run_bass_kernel_spmd

    if "nc" not in _CACHE:
        _CACHE["nc"] = _build()
    nc = _CACHE["nc"]
    in_maps = _shard(inputs)
    res = run_bass_kernel_spmd(nc, in_maps, core_ids=list(range(NC)))
    return _unshard(res.results)


# revision 9
# speedup vs baseline: 1.0091x; 1.0091x over previous
"""Multi-query sparse attention (causal + rel-pos-bias + XL memory) on 8 TRN2 cores.

Sharding: queries are sharded across cores. Core c handles query blocks
A=[64c, 64c+64) and B=[64(15-c), 64(15-c)+64) for both batch elements --
the A/B pairing balances causal work. K/V (single shared head) are computed
from each core's own rows and AllGathered (bf16). rel_pos_bias is sharded by
query rows, host-transposed to [keys, head, query] bf16 layout with the
causal mask baked in as -3e38 (exp -> 0), so the device computes sim directly
in [keys, head*query] layout and attn@v consumes it without any transpose.
Softmax uses exp(sim)*exp(bias) (no max subtraction -- values are small) and
a ones-column in v to get the normalizer from the same matmul. All large
matmuls run in bf16 (fp32 matmul is half-rate LOW_HIGH on trn2); PSUM
accumulation stays fp32. Heads are processed in pairs (even/odd stacked on
partitions) so q/out projections contract over K=128.

The collective has a ~45us barrier+trigger latency, so the program is ordered
to overlap it: kv projection + AllGather issue first, then q projection and
the XL-memory half of attention (keys 0..511, gather-independent, partials
parked in SBUF) run under the collective; the gathered-keys half follows.
"""

import numpy as np

B, N, DIM = 2, 1024, 1024
H, DH = 16, 64
MEM = 512
J = N + MEM  # 1536
NC = 8
SCALE = DH ** -0.5
NEG = -3.0e38

_CACHE = {}

# head-slot permutation: slot i holds head HEAD_PERM[i]
HEAD_PERM = []
for g in range(2):
    HEAD_PERM += [8 * g + 2 * t for t in range(4)] + [8 * g + 2 * t + 1 for t in range(4)]


def _patch_tile_drain():
    """This walrus build only allows one sync-wait per CTRL instruction; the
    stock TileContext final drain carries several. Split them into
    single-wait nops."""
    from concourse import tile
    from concourse.vector_clock import ScopedClock, VectorClock

    if getattr(tile.TileContext, "_drain_patched", False):
        return

    def _drain_and_barrier(self, tick_clock, wait_clock):
        g = tick_clock.global_clock
        n = len(g)
        for p in range(n):
            if g[p] > 0:
                partial = VectorClock([g[i] if i == p else 0 for i in range(n)])
                nop_inst = self.nc.sync.nop()
                wait_clock.add_sem_waits(nop_inst.ins, ScopedClock({None: partial}))
        self.nc.sync.drain()
        self.nc.all_engine_barrier()
        assert self.sems is not None
        popped = self.nc._tile_sem_poison_stack.pop()
        assert popped is self._sem_poison
        self.nc.clear_and_free_semaphores(list(self.sems.allocated().values()))
        self.nc.all_engine_barrier()

    tile.TileContext._drain_and_barrier = _drain_and_barrier
    tile.TileContext._drain_patched = True


def _split_multiwait(nc, mybir):
    """Walrus here allows only one sync-wait per instruction: hoist extra
    waits onto same-engine nops placed immediately before."""
    k = 0
    for bb in nc.main_func.blocks:
        newl = []
        changed = False
        for inst in bb.instructions:
            si = inst.sync_info
            if si is not None and si.on_wait and len(si.on_wait) > 1:
                waits = list(si.on_wait)
                for w in waits[:-1]:
                    nop = mybir.InstNoOp(name=f"wsplit-{k}", ins=[], outs=[])
                    k += 1
                    nop.engine = inst.engine
                    nop.sync_info = mybir.SyncInfo(on_wait=[w], on_update=[])
                    newl.append(nop)
                si.on_wait = [waits[-1]]
                changed = True
            newl.append(inst)
        if changed:
            bb.instructions[:] = newl
            assert len(bb.instructions) == len(newl), "bb.instructions not mutable"


def _build():
    import concourse.bass as bass
    from concourse import mybir, tile

    _patch_tile_drain()
    f32 = mybir.dt.float32
    bf16 = mybir.dt.bfloat16
    AO = mybir.AluOpType
    AF = mybir.ActivationFunctionType

    nc = bass.Bass(target_bir_lowering=False)

    # ---- per-core dram parameters ----
    x_own = nc.declare_dram_parameter("x_own", [4 * 64, DIM], f32, isOutput=False)
    biasT_a = nc.declare_dram_parameter("biasT_a", [N, 2, 8, 64], bf16, isOutput=False)
    biasT_b = nc.declare_dram_parameter("biasT_b", [J, 2, 8, 64], bf16, isOutput=False)
    xl_k = nc.declare_dram_parameter("xl_k", [B, MEM, DH], f32, isOutput=False)
    xl_v = nc.declare_dram_parameter("xl_v", [B, MEM, DH], f32, isOutput=False)
    wq_d = nc.declare_dram_parameter("Wq", [DIM, DIM], f32, isOutput=False)
    wkv_d = nc.declare_dram_parameter("Wkv", [DIM, 2 * DH], f32, isOutput=False)
    wout_d = nc.declare_dram_parameter("Wout", [DIM, DIM], f32, isOutput=False)
    ident_d = nc.declare_dram_parameter("ident", [128, 128], f32, isOutput=False)
    identb_d = nc.declare_dram_parameter("identb", [128, 128], bf16, isOutput=False)
    ones_d = nc.declare_dram_parameter("ones", [128, 64], bf16, isOutput=False)
    bout_d = nc.declare_dram_parameter("bout_bc", [64, DIM], f32, isOutput=False)

    out_part = nc.declare_dram_parameter("out_part", [B, 2, 64, DIM], f32, isOutput=True)
    nxl_part = nc.declare_dram_parameter("nxl_part", [B, 64, 2, DH], f32, isOutput=True)

    NKT = (8, 12)  # key 128-chunks: query block A sees keys<1024, B keys<1536
    biasT = (biasT_a, biasT_b)

    with tile.TileContext(nc, num_cores=NC) as tc:
        import contextlib

        ctx = contextlib.ExitStack()
        with ctx:
            per = ctx.enter_context(tc.tile_pool(name="persist", bufs=1))
            dram = ctx.enter_context(tc.tile_pool(name="dram", bufs=1, space="DRAM"))
            att = ctx.enter_context(tc.tile_pool(name="att", bufs=8))
            biasp = ctx.enter_context(tc.tile_pool(name="bias", bufs=4))
            small = ctx.enter_context(tc.tile_pool(name="small", bufs=4))
            outp = ctx.enter_context(tc.tile_pool(name="outp", bufs=4))
            part = ctx.enter_context(tc.tile_pool(name="part", bufs=1))
            spsum = ctx.enter_context(tc.tile_pool(name="spsum", bufs=2, space="PSUM"))
            opsum = ctx.enter_context(tc.tile_pool(name="opsum", bufs=1, space="PSUM"))
            fpsum = ctx.enter_context(tc.tile_pool(name="fpsum", bufs=1, space="PSUM"))

            # ---------- persistent SBUF ----------
            ident = per.tile([128, 128], f32, tag="ident")
            nc.sync.dma_start(out=ident[:], in_=ident_d[:])
            identb = per.tile([128, 128], bf16, tag="identb")
            nc.sync.dma_start(out=identb[:], in_=identb_d[:])
            ones_sb = per.tile([128, 64], bf16, tag="ones")
            nc.sync.dma_start(out=ones_sb[:], in_=ones_d[:])
            bout_sb = per.tile([64, DIM], f32, tag="bout")
            nc.sync.dma_start(out=bout_sb[:], in_=bout_d[:])
            wout_bf = [per.tile([128, DIM], bf16, name=f"woutb{j}", tag=f"woutb{j}") for j in range(8)]
            qT = per.tile([64, H, 256], bf16, name="qT", tag="qT")
            kT = [per.tile([64, J], bf16, name=f"kT{b}", tag=f"kT{b}") for b in range(B)]
            v_aug = [
                [per.tile([128, DH + 1], bf16, name=f"vaug{b}_{j}", tag=f"vaug{b}_{j}") for j in range(12)]
                for b in range(B)
            ]
            outT2 = [per.tile([128, 8, 64], bf16, name=f"outT2{b}", tag=f"outT2{b}") for b in range(B)]
            # SBUF-parked XL-pass partial accumulators [65, 8h, 64q]
            opo = [
                [
                    [part.tile([65, 8, 64], f32, name=f"opo{q}_{b}_{g}", tag=f"opo{q}_{b}_{g}") for g in range(2)]
                    for b in range(B)
                ]
                for q in range(2)
            ]

            kv_bounce = dram.tile([128, 256], bf16, tag="kv_bounce")
            gathered = dram.tile([NC * 128, 256], bf16, tag="gathered", addr_space="Shared")

            proj = ctx.enter_context(tc.tile_pool(name="proj", bufs=1))

            # ---------- phase 1a: x -> xT, kv projection, issue AllGather ----------
            x_sb = []
            for rt in range(2):
                t = proj.tile([128, DIM], f32, name=f"x{rt}", tag=f"x{rt}")
                nc.sync.dma_start(out=t[:], in_=x_own[128 * rt : 128 * rt + 128, :])
                x_sb.append(t)
            wkv_sb = []
            for kt in range(8):
                t = proj.tile([128, 2 * DH], f32, name=f"wkv{kt}", tag=f"wkv{kt}")
                nc.sync.dma_start(out=t[:], in_=wkv_d[128 * kt : 128 * kt + 128, :])
                wkv_sb.append(t)
            xT_sb = [proj.tile([128, 256], f32, name=f"xT{kt}", tag=f"xT{kt}") for kt in range(8)]
            xT_bf = [proj.tile([128, 256], bf16, name=f"xTb{kt}", tag=f"xTb{kt}") for kt in range(8)]
            for rt in range(2):
                for kt in range(8):
                    pt = spsum.tile([128, 128], f32, name="ptp", tag="ps")
                    nc.tensor.transpose(
                        pt[:], x_sb[rt][:, 128 * kt : 128 * kt + 128], ident[:]
                    )
                    nc.vector.tensor_copy(
                        out=xT_sb[kt][:, 128 * rt : 128 * rt + 128], in_=pt[:]
                    )
                    nc.vector.tensor_copy(
                        out=xT_bf[kt][:, 128 * rt : 128 * rt + 128], in_=pt[:]
                    )

            # kv projection fp32 (feeds new_xl exactly); bf16 copy for gather
            pkv = spsum.tile([128, 256], f32, name="pkv", tag="ps")
            for kt in range(8):
                nc.tensor.matmul(
                    pkv[:], wkv_sb[kt][:], xT_sb[kt][:], start=(kt == 0), stop=(kt == 7)
                )
            kv_sb = proj.tile([128, 256], f32, tag="kv")
            nc.vector.tensor_copy(out=kv_sb[:], in_=pkv[:])
            kv_bf = proj.tile([128, 256], bf16, tag="kvb")
            nc.vector.tensor_copy(out=kv_bf[:], in_=pkv[:])
            nc.sync.dma_start(out=kv_bounce[:], in_=kv_bf[:])

            # ---------- phase 2: AllGather k/v (bf16), issued early ----------
            nc.gpsimd.collective_compute(
                "AllGather",
                mybir.AluOpType.bypass,
                replica_groups=[list(range(NC))],
                ins=[kv_bounce[:]],
                outs=[gathered[:]],
            )

            # ---------- phase 1b: q projection + xl assembly (under collective) --
            wq_bf = []
            for kt in range(8):
                t = proj.tile([128, DIM], f32, name=f"wqf{kt}", tag=f"wqf{kt}")
                nc.sync.dma_start(out=t[:], in_=wq_d[128 * kt : 128 * kt + 128, :])
                tb = proj.tile([128, DIM], bf16, name=f"wqb{kt}", tag=f"wqb{kt}")
                # fold the attention scale into Wq during the cast
                nc.vector.tensor_scalar_mul(tb[:], t[:], SCALE)
                wq_bf.append(tb)
            # q projection in head pairs (K=128), scale pre-folded into Wq.
            for hp in range(8):
                pq = spsum.tile([128, 256], f32, name="pq", tag="ps")
                for kt in range(8):
                    nc.tensor.matmul(
                        pq[:],
                        wq_bf[kt][:, 128 * hp : 128 * hp + 128],
                        xT_bf[kt][:],
                        start=(kt == 0),
                        stop=(kt == 7),
                    )
                g, t = hp // 4, hp % 4
                nc.vector.tensor_copy(out=qT[:, 8 * g + t, :], in_=pq[0:64])
                nc.vector.tensor_copy(out=qT[:, 8 * g + 4 + t, :], in_=pq[64:128])

            # new_xl output: own B-block k,v (cols 64+128b) transposed to rows
            for b in range(B):
                pnx = spsum.tile([64, 2, 64], f32, name="pnx", tag="ps")
                nc.tensor.transpose(
                    pnx[:], kv_sb[:, 64 + 128 * b : 128 + 128 * b], ident[:]
                )
                nx_sb = proj.tile([64, 2, 64], f32, tag="nx")
                nc.vector.tensor_copy(out=nx_sb[:], in_=pnx[:])
                nc.sync.dma_start(out=nxl_part[b], in_=nx_sb[:])

            # xl parts of kT and v_aug (gather-independent)
            for b in range(B):
                for j in range(4):
                    t = proj.tile([128, DH], f32, tag="xlk")
                    nc.sync.dma_start(out=t[:], in_=xl_k[b, 128 * j : 128 * j + 128, :])
                    pt = spsum.tile([64, 128], f32, name="pxlk", tag="ps")
                    nc.tensor.transpose(pt[:], t[:], ident[:])
                    nc.vector.tensor_copy(
                        out=kT[b][:, 128 * j : 128 * j + 128], in_=pt[:]
                    )
                for j in range(4):
                    t = proj.tile([128, DH], f32, tag="xlv")
                    nc.sync.dma_start(out=t[:], in_=xl_v[b, 128 * j : 128 * j + 128, :])
                    nc.vector.tensor_copy(out=v_aug[b][j][:, 0:DH], in_=t[:])
                    nc.sync.dma_start(
                        out=v_aug[b][j][:, DH : DH + 1], in_=ones_d[:, 0:1]
                    )
            # Wout load f32, cast to bf16 pair tiles (only needed at the end)
            for j in range(8):
                t = proj.tile([128, DIM], f32, tag="wof")
                nc.sync.dma_start(out=t[:], in_=wout_d[128 * j : 128 * j + 128, :])
                nc.vector.tensor_copy(out=wout_bf[j][:], in_=t[:])

            # ---------- helper: one attention tile ----------
            def attn_tile(qbi, kt, b, g, po_t, start, stop):
                qc = 64 * (2 * b + qbi)
                ps = spsum.tile([128, 8, 64], f32, name=f"ps{qbi}_{kt}_{b}_{g}", tag="ps")
                nc.tensor.matmul(
                    ps[:],
                    kT[b][:, 128 * kt : 128 * kt + 128],
                    qT[:, 8 * g : 8 * g + 8, qc : qc + 64],
                    start=True,
                    stop=True,
                )
                at = att.tile([128, 8, 64], bf16, name=f"at{qbi}_{kt}_{b}_{g}", tag="attn")
                nc.scalar.activation(out=at[:], in_=ps[:], func=AF.Exp)
                nc.vector.scalar_tensor_tensor(
                    out=at[:], in0=at[:], scalar=1.0, in1=ebias[g][:],
                    op0=AO.mult, op1=AO.mult,
                )
                nc.tensor.matmul(
                    po_t[:], v_aug[b][kt][:], at[:], start=start, stop=stop
                )

            # ---------- phase 4a: XL-keys attention (kt 0..3), park partials ----
            for qbi in range(2):
                po = [
                    [opsum.tile([65, 8, 64], f32, name=f"poa{qbi}_{b}_{g}", tag=f"po{b}_{g}") for g in range(2)]
                    for b in range(B)
                ]
                for kt in range(4):
                    bt = biasp.tile([128, 2, 8, 64], bf16, name=f"bta{qbi}_{kt}", tag="bias")
                    nc.sync.dma_start(out=bt[:], in_=biasT[qbi][128 * kt : 128 * kt + 128])
                    ebias = [att.tile([128, 8, 64], bf16, name=f"eba{qbi}_{kt}_{g}", tag="ebias") for g in range(2)]
                    for g in range(2):
                        nc.scalar.activation(out=ebias[g][:], in_=bt[:, g], func=AF.Exp)
                    for b in range(B):
                        for g in range(2):
                            attn_tile(qbi, kt, b, g, po[b][g], kt == 0, kt == 3)
                for b in range(B):
                    for g in range(2):
                        nc.vector.tensor_copy(out=opo[qbi][b][g][:], in_=po[b][g][:])

            # ---------- phase 3b: gathered kT / v_aug assembly ----------
            for b in range(B):
                for m in range(16):
                    cm = m if m < 8 else 15 - m
                    off = (0 if m < 8 else 64) + 128 * b
                    nc.sync.dma_start(
                        out=kT[b][:, 512 + 64 * m : 576 + 64 * m],
                        in_=gathered[128 * cm : 128 * cm + 64, off : off + 64],
                    )
                for j in range(4, 12):
                    vt = proj.tile([64, 128], bf16, tag="vt")
                    for mm in range(2):
                        m = 2 * (j - 4) + mm
                        cm = m if m < 8 else 15 - m
                        off = (0 if m < 8 else 64) + 128 * b
                        nc.sync.dma_start(
                            out=vt[:, 64 * mm : 64 * mm + 64],
                            in_=gathered[128 * cm + 64 : 128 * cm + 128, off : off + 64],
                        )
                    pv = spsum.tile([128, 64], bf16, name="pv", tag="ps")
                    nc.tensor.transpose(pv[:], vt[:], identb[0:64, 0:64])
                    nc.vector.tensor_copy(out=v_aug[b][j][:, 0:DH], in_=pv[:])
                    nc.sync.dma_start(
                        out=v_aug[b][j][:, DH : DH + 1], in_=ones_d[:, 0:1]
                    )

            # ---------- phase 4b: gathered-keys attention + merge + output ------
            for qbi in range(2):
                nkt = NKT[qbi]
                po = [
                    [opsum.tile([65, 8, 64], f32, name=f"pob{qbi}_{b}_{g}", tag=f"po{b}_{g}") for g in range(2)]
                    for b in range(B)
                ]
                for kt in range(4, nkt):
                    bt = biasp.tile([128, 2, 8, 64], bf16, name=f"btb{qbi}_{kt}", tag="bias")
                    nc.sync.dma_start(out=bt[:], in_=biasT[qbi][128 * kt : 128 * kt + 128])
                    ebias = [att.tile([128, 8, 64], bf16, name=f"ebb{qbi}_{kt}_{g}", tag="ebias") for g in range(2)]
                    for g in range(2):
                        nc.scalar.activation(out=ebias[g][:], in_=bt[:, g], func=AF.Exp)
                    for b in range(B):
                        for g in range(2):
                            attn_tile(qbi, kt, b, g, po[b][g], kt == 4, kt == nkt - 1)
                # merge partials, normalize (1/Z = exp(-ln Z)), write outT2 bf16
                for b in range(B):
                    for g in range(2):
                        sm = att.tile([65, 8, 64], f32, name=f"sm{qbi}_{b}_{g}", tag="sum")
                        nc.vector.scalar_tensor_tensor(
                            out=sm[:], in0=po[b][g][:], scalar=1.0,
                            in1=opo[qbi][b][g][:], op0=AO.mult, op1=AO.add,
                        )
                        lnz = small.tile([1, 8, 64], f32, tag="lnz")
                        nc.scalar.activation(out=lnz[:], in_=sm[64:65], func=AF.Ln)
                        rz = small.tile([1, 8, 64], bf16, tag="rz")
                        nc.scalar.activation(out=rz[:], in_=lnz[:], func=AF.Exp, scale=-1.0)
                        rzp = fpsum.tile([64, 8, 64], f32, tag="pb")
                        nc.tensor.matmul(
                            rzp[:], ones_sb[0:1, :], rz[:], start=True, stop=True
                        )
                        rzb = small.tile([64, 8, 64], f32, tag="rzb")
                        nc.vector.tensor_copy(out=rzb[:], in_=rzp[:])
                        nc.vector.scalar_tensor_tensor(
                            out=outT2[b][0:64, 4 * g : 4 * g + 4, :],
                            in0=sm[0:64, 0:4, :], scalar=1.0,
                            in1=rzb[0:64, 0:4, :], op0=AO.mult, op1=AO.mult,
                        )
                        nc.vector.scalar_tensor_tensor(
                            out=outT2[b][64:128, 4 * g : 4 * g + 4, :],
                            in0=sm[0:64, 4:8, :], scalar=1.0,
                            in1=rzb[0:64, 4:8, :], op0=AO.mult, op1=AO.mult,
                        )
                # final projection (K=128 head pairs) + bias
                for b in range(B):
                    for nh in range(2):
                        pf = fpsum.tile([64, 512], f32, tag="pf")
                        for j in range(8):
                            nc.tensor.matmul(
                                pf[:],
                                outT2[b][:, j, :],
                                wout_bf[j][:, 512 * nh : 512 * nh + 512],
                                start=(j == 0),
                                stop=(j == 7),
                            )
                        ot = outp.tile([64, 512], f32, tag="ot")
                        nc.vector.scalar_tensor_tensor(
                            out=ot[:], in0=pf[:], scalar=1.0,
                            in1=bout_sb[:, 512 * nh : 512 * nh + 512],
                            op0=AO.mult, op1=AO.add,
                        )
                        nc.sync.dma_start(
                            out=out_part[b, qbi, :, 512 * nh : 512 * nh + 512],
                            in_=ot[:],
                        )
    _split_multiwait(nc, mybir)
    return nc


def _shard(inputs):
    from concourse import mybir

    bfdt = mybir.dt.np(mybir.dt.bfloat16)
    x = np.asarray(inputs["x"], dtype=np.float32)
    xlm = np.asarray(inputs["xl_memory"], dtype=np.float32)
    bias = np.asarray(inputs["rel_pos_bias"], dtype=np.float32)
    Wq = np.ascontiguousarray(np.asarray(inputs["Wq"], dtype=np.float32))
    Wkv = np.ascontiguousarray(np.asarray(inputs["Wkv"], dtype=np.float32))
    Wout = np.ascontiguousarray(np.asarray(inputs["Wout"], dtype=np.float32))
    bout = np.asarray(inputs["bout"], dtype=np.float32)

    ident = np.eye(128, dtype=np.float32)
    identb = np.eye(128, dtype=np.float32).astype(bfdt)
    ones = np.ones((128, 64), dtype=np.float32).astype(bfdt)
    bout_bc = np.ascontiguousarray(np.broadcast_to(bout, (64, DIM)))
    xl_k = np.ascontiguousarray(xlm[:, :, 0, :])
    xl_v = np.ascontiguousarray(xlm[:, :, 1, :])

    jj = np.arange(J)[:, None]  # keys (concat space)
    rr = np.arange(64)[None, :]

    in_maps = []
    for c in range(NC):
        qsA, qsB = 64 * c, 64 * (15 - c)
        x_own = np.concatenate(
            [x[0, qsA : qsA + 64], x[0, qsB : qsB + 64],
             x[1, qsA : qsA + 64], x[1, qsB : qsB + 64]], axis=0,
        )
        bT = []
        for qs, klen in ((qsA, N), (qsB, J)):
            bb = bias[:, qs : qs + 64, :klen]  # [16, 64, klen]
            bb = np.transpose(bb, (2, 0, 1)).copy()  # [klen, 16, 64]
            bb = bb[:, HEAD_PERM, :]  # head-slot order [evens, odds] per group
            m = jj[:klen] > (qs + rr + 512)  # [klen, 64] causal+pad mask
            bb[m[:, None, :].repeat(H, axis=1)] = NEG
            bT.append(np.ascontiguousarray(bb.reshape(klen, 2, 8, 64).astype(bfdt)))
        in_maps.append(
            {
                "x_own": np.ascontiguousarray(x_own),
                "biasT_a": bT[0],
                "biasT_b": bT[1],
                "xl_k": xl_k,
                "xl_v": xl_v,
                "Wq": Wq,
                "Wkv": Wkv,
                "Wout": Wout,
                "ident": ident,
                "identb": identb,
                "ones": ones,
                "bout_bc": bout_bc,
            }
        )
    return in_maps


def _unshard(results):
    out = np.zeros((B, N, DIM), dtype=np.float32)
    new_xl = np.zeros((B, MEM, 2, DH), dtype=np.float32)
    for c in range(NC):
        qsA, qsB = 64 * c, 64 * (15 - c)
        op = results[c]["out_part"]
        out[:, qsA : qsA + 64] = op[:, 0]
        out[:, qsB : qsB + 64] = op[:, 1]
        new_xl[:, qsB - 512 : qsB - 512 + 64] = results[c]["nxl_part"]
    return out, new_xl


def kernel(**inputs):
    from concourse.bass_utils import run_bass_kernel_spmd

    if "nc" not in _CACHE:
        _CACHE["nc"] = _build()
    nc = _CACHE["nc"]
    in_maps = _shard(inputs)
    res = run_bass_kernel_spmd(nc, in_maps, core_ids=list(range(NC)))
    return _unshard(res.results)
